# revision 12
# baseline (speedup 1.0000x reference)
"""GAU (gated attention unit) Trainium2 Bass kernel.

Sharding: data-parallel over batch (4) x tensor-parallel over hidden (2).
Core c handles batch c//2 and hidden half c%2 (1024 of 2048 channels of
v/gate plus the matching rows of W_out). Each core produces a partial
output [N, DIM]; the host sums each pair and adds b_out.

Math notes:
 - LayerNorm affine (ln_g, ln_b) is folded into the weights host-side.
   LN is scale-invariant, so feeding 0.5*x (exact in bf16: pure exponent
   shift) gives the same normalized z, and both cores of a pair add
   0.5*x for the residual (summing to x exactly).
 - sim/(i+1) then relu()**2: relu(s/c)^2 = relu(s)^2/c^2 for c>0, so the
   1/(i+1)^2 factor commutes through the (linear) attention*V, gate
   multiply and output projection, and is applied as a per-row scale on
   the final partial output.
 - Attention is computed transposed (S^T[j,i] blocks) so that relu^2 and
   the causal mask are plain elementwise ops and A^T tiles directly feed
   the PV matmuls as stationary operands.
"""

import numpy as np

B, N, DIM, QK, HID = 4, 4096, 1024, 128, 2048
HH = HID // 2
NCORES = 8

_NC_CACHE = {}


def _build_nc(n, dim, qk, hh, rblk, ah, num_devices, use_silu=True):
    """Build the single-core Bass/Tile program (same program on all cores)."""
    from contextlib import ExitStack

    import concourse.mybir as mybir
    import concourse.tile as tile
    from concourse import bacc
    from concourse.masks import make_identity

    f32 = mybir.dt.float32
    bf16 = mybir.dt.bfloat16
    AF = mybir.ActivationFunctionType
    OP = mybir.AluOpType

    nrc = n // 128        # 128-row chunks
    nd = dim // 128       # contraction chunks
    nRb = n // rblk       # big row blocks
    spr = rblk // 128     # 128-row subchunks per R block
    hpr = rblk // ah      # attention column blocks per R block
    nhc = hh // 128       # hidden chunks (per-core half)
    nvh = hh // 512       # 512-wide hidden column groups
    ndc = dim // 512      # 512-wide dim column groups
    gsz = min(512, dim)   # bn_stats subgroup size
    ng = dim // gsz
    assert qk == 128 and rblk % ah == 0 and ah % 128 == 0

    nc = bacc.Bacc("TRN2", target_bir_lowering=False, debug=False,
                   num_devices=num_devices)

    xh_d = nc.dram_tensor("xh", [n, dim], bf16, kind="ExternalInput").ap()
    wv_d = nc.dram_tensor("wv", [dim, hh], bf16, kind="ExternalInput").ap()
    wg_d = nc.dram_tensor("wg", [dim, hh], bf16, kind="ExternalInput").ap()
    wqk_d = nc.dram_tensor("wqk", [dim, qk], bf16, kind="ExternalInput").ap()
    wout_d = nc.dram_tensor("wout", [hh, dim], bf16, kind="ExternalInput").ap()
    g0_d = nc.dram_tensor("g0", [qk, 1], f32, kind="ExternalInput").ap()
    b0_d = nc.dram_tensor("b0", [qk, 1], f32, kind="ExternalInput").ap()
    g1_d = nc.dram_tensor("g1", [qk, 1], f32, kind="ExternalInput").ap()
    b1_d = nc.dram_tensor("b1", [qk, 1], f32, kind="ExternalInput").ap()
    inv_d = nc.dram_tensor("inv2", [128, nrc], f32, kind="ExternalInput").ap()
    mask_d = nc.dram_tensor("mask", [128, 2, ah], bf16, kind="ExternalInput").ap()
    out_d = nc.dram_tensor("outp", [n, dim], bf16, kind="ExternalOutput").ap()

    with tile.TileContext(nc) as tc, ExitStack() as ctx:
        p = lambda name, bufs, **kw: ctx.enter_context(
            tc.tile_pool(name=name, bufs=bufs, **kw))
        const = p("const", 1)
        wpool = p("w", 1)
        vpool = p("v", nrc)
        kpool = p("k", nRb)
        xpool = p("x", 2 * spr - 2)
        zpool = p("z", 2)
        ztp = p("zt", nd + 2)
        qpool = p("q", 2)
        gpool = p("g", nhc + 2)
        atpool = p("at", nrc + 2)
        gopool = p("go", hpr * nhc + 2)
        spool = p("s", 4)
        rpool = p("r", 4)
        opool = p("osb", 2)
        psum = p("ps", 6, space="PSUM")
        tpsum = p("tps", 2, space="PSUM")

        def silu_act(out_ap, psum_ap, shape):
            # CoreSim lacks Silu; hardware has it in the ACT LUT.
            if use_silu:
                nc.scalar.activation(out_ap, psum_ap, AF.Silu)
            else:
                sg = spool.tile(shape, bf16, name="sg", tag="sg")
                nc.scalar.activation(sg, psum_ap, AF.Sigmoid)
                nc.vector.tensor_mul(out_ap, sg, psum_ap)

        ident = const.tile([128, 128], bf16)
        make_identity(nc, ident)
        eps_t = const.tile([128, 1], f32)
        nc.vector.memset(eps_t, 1e-5)
        g0 = const.tile([qk, 1], f32)
        nc.sync.dma_start(g0, g0_d)
        b0 = const.tile([qk, 1], f32)
        nc.sync.dma_start(b0, b0_d)
        g1 = const.tile([qk, 1], f32)
        nc.sync.dma_start(g1, g1_d)
        b1 = const.tile([qk, 1], f32)
        nc.sync.dma_start(b1, b1_d)
        inv2 = const.tile([128, nrc], f32)
        nc.sync.dma_start(inv2, inv_d)
        masks = const.tile([128, 2, ah], bf16)
        nc.sync.dma_start(masks, mask_d)

        wv = []
        wg = []
        wqk = []
        wout = []
        for d in range(nd):
            wv_t = wpool.tile([128, hh], bf16, name=f"wv{d}", tag=f"wv{d}")
            nc.sync.dma_start(wv_t, wv_d[d * 128:(d + 1) * 128, :])
            wv.append(wv_t)
            wg_t = wpool.tile([128, hh], bf16, name=f"wg{d}", tag=f"wg{d}")
            nc.sync.dma_start(wg_t, wg_d[d * 128:(d + 1) * 128, :])
            wg.append(wg_t)
            wqk_t = wpool.tile([128, qk], bf16, name=f"wqk{d}", tag=f"wqk{d}")
            nc.sync.dma_start(wqk_t, wqk_d[d * 128:(d + 1) * 128, :])
            wqk.append(wqk_t)
        for hc in range(nhc):
            wo_t = wpool.tile([128, dim], bf16, name=f"wo{hc}", tag=f"wo{hc}")
            nc.sync.dma_start(wo_t, wout_d[hc * 128:(hc + 1) * 128, :])
            wout.append(wo_t)

        v_tiles = [None] * nrc
        kT = [None] * nRb

        for Rb in range(nRb):
            R0 = Rb * rblk
            # ---- LayerNorm + transpose to zT ----
            zT = [ztp.tile([128, rblk], bf16, name=f"zT{Rb}_{d}", tag="zT")
                  for d in range(nd)]
            x_cur = []
            for s in range(spr):
                rc = Rb * spr + s
                xt = xpool.tile([128, dim], bf16, name=f"x{rc}", tag="x")
                nc.sync.dma_start(xt, xh_d[rc * 128:(rc + 1) * 128, :])
                x_cur.append(xt)
                stats = spool.tile([128, ng, 6], f32, tag="st")
                for g in range(ng):
                    nc.vector.bn_stats(stats[:, g, :], xt[:, g * gsz:(g + 1) * gsz])
                mv = spool.tile([128, 2], f32, tag="mv")
                nc.vector.bn_aggr(mv, stats)
                sd = spool.tile([128, 1], f32, tag="sd")
                nc.scalar.activation(sd, mv[:, 1:2], AF.Sqrt, bias=eps_t)
                rstd = spool.tile([128, 1], f32, tag="rs")
                nc.vector.reciprocal(rstd, sd)
                nm = spool.tile([128, 1], f32, tag="nm")
                nc.vector.tensor_scalar(nm, mv[:, 0:1], rstd, -1.0,
                                        OP.mult, OP.mult)
                zrow = zpool.tile([128, dim], bf16, tag="z")
                nc.scalar.activation(zrow, xt, AF.Identity, bias=nm, scale=rstd)
                for d in range(nd):
                    tp = tpsum.tile([128, 128], bf16, tag="tp")
                    nc.tensor.transpose(tp, zrow[:, d * 128:(d + 1) * 128], ident)
                    nc.scalar.copy(zT[d][:, s * 128:(s + 1) * 128], tp)

            # ---- q/k projection (transposed) ----
            qk_ps = psum.tile([128, rblk], f32, tag="mm")
            for d in range(nd):
                nc.tensor.matmul(qk_ps, lhsT=wqk[d], rhs=zT[d],
                                 start=(d == 0), stop=(d == nd - 1))
            silu_t = spool.tile([128, rblk], bf16, tag="sl", bufs=2)
            silu_act(silu_t, qk_ps, [128, rblk])
            qT = qpool.tile([128, rblk], bf16, tag="qT")
            nc.scalar.activation(qT, silu_t, AF.Identity, bias=b0, scale=g0)
            kt = kpool.tile([128, rblk], bf16, name=f"kT{Rb}", tag="kT")
            nc.scalar.activation(kt, silu_t, AF.Identity, bias=b1, scale=g1)
            kT[Rb] = kt

            # ---- v rows (row-major) ----
            for s in range(spr):
                rc = Rb * spr + s
                vt = vpool.tile([128, hh], bf16, name=f"v{rc}", tag="v")
                for h5 in range(nvh):
                    v_ps = psum.tile([128, 512], f32, tag="mm")
                    for d in range(nd):
                        nc.tensor.matmul(
                            v_ps, lhsT=zT[d][:, s * 128:(s + 1) * 128],
                            rhs=wv[d][:, h5 * 512:(h5 + 1) * 512],
                            start=(d == 0), stop=(d == nd - 1))
                    silu_act(vt[:, h5 * 512:(h5 + 1) * 512], v_ps, [128, 512])
                v_tiles[rc] = vt

            # ---- gate (transposed) ----
            gT = []
            for hc in range(nhc):
                g_ps = psum.tile([128, rblk], f32, tag="mm")
                for d in range(nd):
                    nc.tensor.matmul(g_ps, lhsT=wg[d][:, hc * 128:(hc + 1) * 128],
                                     rhs=zT[d], start=(d == 0), stop=(d == nd - 1))
                gt = gpool.tile([128, rblk], bf16, name=f"gT{Rb}_{hc}", tag="gT")
                silu_act(gt, g_ps, [128, rblk])
                gT.append(gt)

            # ---- attention (transposed blocks) + gate ----
            goT = [[None] * nhc for _ in range(hpr)]
            for half in range(hpr):
                i0 = R0 + half * ah
                ncb = (i0 + ah) // 128
                at_tiles = []
                for cb in range(ncb):
                    st_ps = psum.tile([128, ah], f32, tag="mm")
                    nc.tensor.matmul(
                        st_ps,
                        lhsT=kT[cb // spr][:, (cb % spr) * 128:(cb % spr + 1) * 128],
                        rhs=qT[:, half * ah:(half + 1) * ah],
                        start=True, stop=True)
                    rl = rpool.tile([128, ah], bf16, tag="rl")
                    nc.scalar.activation(rl, st_ps, AF.Relu)
                    at = atpool.tile([128, ah], bf16, name=f"at{Rb}_{half}_{cb}",
                                     tag="at")
                    if cb >= ncb - 2:
                        sq = rpool.tile([128, ah], bf16, tag="sq")
                        nc.vector.tensor_mul(sq, rl, rl)
                        nc.vector.tensor_mul(at, sq, masks[:, cb - (ncb - 2), :])
                    else:
                        nc.vector.tensor_mul(at, rl, rl)
                    at_tiles.append(at)
                for hc in range(nhc):
                    ot_ps = psum.tile([128, ah], f32, tag="mm")
                    for cb in range(ncb):
                        nc.tensor.matmul(
                            ot_ps, lhsT=v_tiles[cb][:, hc * 128:(hc + 1) * 128],
                            rhs=at_tiles[cb],
                            start=(cb == 0), stop=(cb == ncb - 1))
                    got = gopool.tile([128, ah], bf16, name=f"go{Rb}_{half}_{hc}",
                                      tag="go")
                    nc.vector.tensor_mul(got, ot_ps,
                                         gT[hc][:, half * ah:(half + 1) * ah])
                    goT[half][hc] = got

            # ---- output projection + residual ----
            for s in range(spr):
                rc = Rb * spr + s
                half = (s * 128) // ah
                io = (s * 128) % ah
                osb = opool.tile([128, dim], bf16, tag="ou", bufs=2)
                for dc in range(ndc):
                    o_ps = psum.tile([128, 512], f32, tag="mm")
                    for hc in range(nhc):
                        nc.tensor.matmul(
                            o_ps, lhsT=goT[half][hc][:, io:io + 128],
                            rhs=wout[hc][:, dc * 512:(dc + 1) * 512],
                            start=(hc == 0), stop=(hc == nhc - 1))
                    ot = rpool.tile([128, 512], bf16, tag="ot", bufs=2)
                    nc.scalar.activation(ot, o_ps, AF.Copy,
                                         scale=inv2[:, rc:rc + 1])
                    nc.vector.tensor_add(
                        osb[:, dc * 512:(dc + 1) * 512], ot,
                        x_cur[s][:, dc * 512:(dc + 1) * 512])
                nc.sync.dma_start(out_d[rc * 128:(rc + 1) * 128, :], osb)

    nc.finalize()
    return nc


def get_nc(n=N, dim=DIM, qk=QK, hh=HH, rblk=512, ah=256, num_devices=NCORES,
           use_silu=True):
    key = (n, dim, qk, hh, rblk, ah, num_devices, use_silu)
    if key not in _NC_CACHE:
        _NC_CACHE[key] = _build_nc(*key)
    return _NC_CACHE[key]


def make_core_inputs(x, ln_g, ln_b, W_hidden, b_hidden, W_qk, b_qk, os_gamma,
                     os_beta, W_out, b_out, n=N, dim=DIM, qk=QK, hh=HH, ah=256,
                     ncores=NCORES):
    """Host-side prep: fold LN affine into weights, cast, shard per core."""
    import ml_dtypes
    bf = ml_dtypes.bfloat16
    x = np.asarray(x, np.float32)
    ln_g = np.asarray(ln_g, np.float32)
    Wh = (np.asarray(W_hidden, np.float32) * ln_g[:, None]).astype(bf)
    Wq = (np.asarray(W_qk, np.float32) * ln_g[:, None]).astype(bf)
    Wo = np.asarray(W_out, np.float32)
    hid = Wh.shape[1] // 2

    nrc = n // 128
    inv2 = ((np.arange(n, dtype=np.float64) + 1.0) ** -2.0).astype(np.float32)
    inv2 = np.ascontiguousarray(inv2.reshape(nrc, 128).T)
    j = np.arange(128)[:, None, None]
    o = (np.arange(2) * 128)[None, :, None]
    i = np.arange(ah)[None, None, :]
    mask = ((j + o) <= i).astype(bf)

    in_maps = []
    for c in range(ncores):
        b, h = c // 2, c % 2
        in_maps.append({
            "xh": (0.5 * x[b]).astype(bf),
            "wv": np.ascontiguousarray(Wh[:, h * hh:(h + 1) * hh]),
            "wg": np.ascontiguousarray(Wh[:, hid + h * hh: hid + (h + 1) * hh]),
            "wqk": Wq,
            "wout": np.ascontiguousarray(Wo[h * hh:(h + 1) * hh, :]).astype(bf),
            "g0": np.asarray(os_gamma[0], np.float32).reshape(qk, 1),
            "b0": np.asarray(os_beta[0], np.float32).reshape(qk, 1),
            "g1": np.asarray(os_gamma[1], np.float32).reshape(qk, 1),
            "b1": np.asarray(os_beta[1], np.float32).reshape(qk, 1),
            "inv2": inv2,
            "mask": mask,
        })
    return in_maps


def _numpy_fallback(x, ln_g, ln_b, W_hidden, b_hidden, W_qk, b_qk, os_gamma,
                    os_beta, W_out, b_out):
    x = np.asarray(x, np.float32)
    mu = x.mean(-1, keepdims=True)
    var = x.var(-1, keepdims=True)
    normed = (x - mu) / np.sqrt(var + 1e-5) * ln_g + ln_b
    hidden = normed @ W_hidden + b_hidden
    hidden = hidden / (1.0 + np.exp(-hidden))
    v, gate = hidden[..., :HID], hidden[..., HID:]
    qkp = normed @ W_qk + b_qk
    qkp = qkp / (1.0 + np.exp(-qkp))
    q = qkp * os_gamma[0] + os_beta[0]
    k = qkp * os_gamma[1] + os_beta[1]
    n = x.shape[-2]
    den = (np.arange(n, dtype=np.float32) + 1.0)[:, None]
    out = np.empty_like(x)
    for bi in range(x.shape[0]):
        sim = q[bi] @ k[bi].T / den
        np.maximum(sim, 0.0, out=sim)
        np.square(sim, out=sim)
        sim *= np.tri(n, dtype=np.float32)
        out[bi] = ((sim @ v[bi]) * gate[bi]) @ W_out + b_out + x[bi]
    return out


def kernel(x, ln_g, ln_b, W_hidden, b_hidden, W_qk, b_qk, os_gamma, os_beta,
           W_out, b_out):
    args = (x, ln_g, ln_b, W_hidden, b_hidden, W_qk, b_qk, os_gamma, os_beta,
            W_out, b_out)
    # b_hidden/b_qk/ln_b sit in positions the device program folds away only
    # when they are zero (always true for this problem's setup_inputs).
    if (np.any(np.asarray(b_hidden)) or np.any(np.asarray(b_qk))
            or np.any(np.asarray(ln_b))):
        return _numpy_fallback(*args)

    from concourse import bass_utils
    nc = get_nc()
    in_maps = make_core_inputs(*args)
    res = bass_utils.run_bass_kernel_spmd(nc, in_maps,
                                          core_ids=list(range(NCORES)))
    parts = [np.asarray(r["outp"], np.float32) for r in res.results]
    out = np.empty((B, N, DIM), np.float32)
    for b in range(B):
        out[b] = parts[2 * b] + parts[2 * b + 1]
    out += np.asarray(b_out, np.float32)
    return out
